# revision 29
# baseline (speedup 1.0000x reference)
"""CBOW negative-sampling loss kernel for 8 Trainium2 NeuronCores.

Math (faithful to the reference, including its [B]+[B,1] broadcast bug):
    c_b   = mean_w ctx_w[context[b, w]]               # [D]
    pos_b = log_sigmoid(emb_w[target[b]] . c_b)
    neg_b = sum_k log_sigmoid(emb_w[noise[b, k]] . c_b)
    out   = -(mean_b pos_b + mean_b neg_b) = -(sum_b (pos_b + neg_b)) / B

Strategy: shard B across the 8 cores (2048 samples each); tables cast to bf16
on the host (halves the random-gather HBM traffic; the dots are ~1e-4 so bf16
is far inside the fp32 reference envelope). Per core, 16 blocks of 128
samples (partition = sample-in-block), processed as 9 chunks of 1-2 blocks.
Per chunk two indirect gathers run on the Pool SWDGE queue (small enough that
the SWDGE descriptor ring never stalls the Pool engine; tiny first chunks
prime the compute pipeline):
  - ctx rows land [word u][block b][D]; context pooling is 10
    PSUM-accumulating identity matmuls per chunk (constant lhsT, each c_ps
    tile owns a full 2KB PSUM bank); the scalar engine downcasts c to bf16.
  - target+noise rows land [block][tgt, noise*10][D].
Dots run on DVE: one bf16 multiply per chunk against broadcast c (2x mode),
then per chunk-group a chain of 2x tensor_tensor folds (128->64->32->16) and
a short 1x tensor_reduce to fp32 (the fold chain replaces most of the
1x-only tensor_reduce). Sigmoid/Ln activation tables are preloaded at t=0 by
dummy activations so the tail pass (Sigmoid(x/10) + Ln with accum_out giving
per-partition sums) costs ~1us. The host sums the per-core partials and
scales by -1/B.
"""

import numpy as np

V, D = 100000, 128
B, W, K = 16384, 10, 10
NCORES = 8
P = 128
B_LOCAL = B // NCORES  # 2048
NBLK = B_LOCAL // P  # 16 blocks of 128 samples
KP1 = K + 1  # 11 emb rows per sample

# Chunks = gather granularity (blocks per indirect-DMA pair). Small first
# chunks prime the compute pipeline; wider later chunks amortize the
# per-matmul fixed cost (LDWEIGHTS + pipeline drain) of the pooling.
CHUNKS = [1, 1, 2, 4, 4, 4]
# Fold groups (indices into CHUNKS): folds/reduce batch several chunks to
# amortize DVE instruction overhead.
GROUPS = [(0, 1, 2), (3,), (4,), (5,)]
assert sum(CHUNKS) == NBLK

_LAST_RESULTS = None  # test harness introspection (exec_time_ns etc.)


def _build_bass(vocab, debug_dots=False):
    import concourse.bass as bass
    import concourse.tile as tile
    from concourse import bacc, mybir

    total_cols = NBLK * W + NBLK * KP1  # 336
    nc = bacc.Bacc(None, target_bir_lowering=False)
    dots_d = (
        nc.declare_dram_parameter(
            "dots", [P, NBLK * KP1], mybir.dt.float32, isOutput=True
        )
        if debug_dots
        else None
    )
    idx_d = nc.declare_dram_parameter(
        "idx", [P, total_cols], mybir.dt.int32, isOutput=False
    )
    ident_d = nc.declare_dram_parameter(
        "ident", [P, P], mybir.dt.bfloat16, isOutput=False
    )
    ctx_w_d = nc.declare_dram_parameter(
        "ctx_w", [vocab, D], mybir.dt.bfloat16, isOutput=False
    )
    emb_w_d = nc.declare_dram_parameter(
        "emb_w", [vocab, D], mybir.dt.bfloat16, isOutput=False
    )
    out_d = nc.declare_dram_parameter("out", [P, 1], mybir.dt.float32, isOutput=True)

    cbo = np.cumsum([0] + CHUNKS).tolist()  # chunk block offsets

    with tile.TileContext(nc) as tc:
        with (
            tc.tile_pool(name="const", bufs=1) as cpool,
            tc.tile_pool(name="work", bufs=2) as wpool,
            tc.tile_pool(name="psum", bufs=3, space="PSUM") as ppool,
        ):
            idx_sb = cpool.tile([P, total_cols], mybir.dt.int32)
            nc.sync.dma_start(out=idx_sb[:], in_=idx_d[:])
            ident_sb = cpool.tile([P, P], mybir.dt.bfloat16)
            nc.sync.dma_start(out=ident_sb[:], in_=ident_d[:])
            all_dots = cpool.tile([P, NBLK * KP1], mybir.dt.float32)
            acc = cpool.tile([P, 1], mybir.dt.float32)

            # Preload Sigmoid/Ln activation tables at t=0 (no deps) so the
            # tail pass doesn't pay two serial ~1.3us table loads.
            dummy = cpool.tile([P, 8], mybir.dt.float32)
            nc.vector.memset(dummy[:], 1.0)
            nc.scalar.activation(
                out=dummy[:], in_=dummy[:], func=mybir.ActivationFunctionType.Sigmoid
            )
            nc.scalar.activation(
                out=dummy[:], in_=dummy[:], func=mybir.ActivationFunctionType.Ln
            )
            # ~3.4us of back-to-back dummy matmuls at t=0: trips the PE HAM
            # activity window so the pooling matmuls run at 2.4GHz, hidden
            # under the initial idx DMA + first gathers.
            warm = ppool.tile([P, 4 * D], mybir.dt.float32, tag="warm")
            for w in range(10):
                nc.tensor.matmul(
                    warm[:, :P],
                    lhsT=ident_sb[:],
                    rhs=ident_sb[:],
                    start=(w == 0),
                    stop=(w == 9),
                )

            # Per-chunk persistent tiles (no recycling -> no false deps).
            c_sbs, Tctxs, Tembs = [], [], []
            for i, nb in enumerate(CHUNKS):
                c_sb = cpool.tile([P, nb * D], mybir.dt.bfloat16, tag=f"c{i}")
                c_sbs.append(c_sb)
                Tctx = cpool.tile([P, W * nb * D], mybir.dt.bfloat16, tag=f"C{i}")
                Tctxs.append(Tctx)
                Temb = cpool.tile([P, nb * KP1 * D], mybir.dt.bfloat16, tag=f"T{i}")
                Tembs.append(Temb)
            prods = []
            for gi, g in enumerate(GROUPS):
                gnb = sum(CHUNKS[i] for i in g)
                prod = wpool.tile(
                    [P, gnb * KP1 * D], mybir.dt.bfloat16, tag=f"prod{gi}"
                )
                prods.append(prod)

            # All gathers up front; the SWDGE queue drains them back to back.
            col = 0
            for i, nb in enumerate(CHUNKS):
                ctx_cols = W * nb
                emb_cols = KP1 * nb
                nc.gpsimd.indirect_dma_start(
                    out=Tctxs[i][:],
                    out_offset=None,
                    in_=ctx_w_d[:],
                    in_offset=bass.IndirectOffsetOnAxis(
                        ap=idx_sb[:, col : col + ctx_cols], axis=0
                    ),
                )
                col += ctx_cols
                nc.gpsimd.indirect_dma_start(
                    out=Tembs[i][:],
                    out_offset=None,
                    in_=emb_w_d[:],
                    in_offset=bass.IndirectOffsetOnAxis(
                        ap=idx_sb[:, col : col + emb_cols], axis=0
                    ),
                )
                col += emb_cols

            # Context pooling: 10 PSUM-accumulating identity matmuls per
            # chunk (c[s, :] = sum_u Tctx[s, slot u]); constant lhsT.
            for i, nb in enumerate(CHUNKS):
                # Full 2KB PSUM bank per tile: matmul start=True resets at
                # bank granularity, so tiles must never share a bank.
                c_ps = ppool.tile([P, 4 * D], mybir.dt.float32, tag="cps")
                for u in range(W):
                    nc.tensor.matmul(
                        c_ps[:, : nb * D],
                        lhsT=ident_sb[:],
                        rhs=Tctxs[i][:, u * nb * D : (u + 1) * nb * D],
                        start=(u == 0),
                        stop=(u == W - 1),
                    )
                nc.scalar.activation(
                    out=c_sbs[i][:],
                    in_=c_ps[:, : nb * D],
                    func=mybir.ActivationFunctionType.Copy,
                )

            # Dots: multiply per chunk (2x), fold chain + reduce per group.
            for gi, g in enumerate(GROUPS):
                prod = prods[gi]
                gnb = sum(CHUNKS[i] for i in g)
                poff = 0
                for i in g:
                    nb = CHUNKS[i]
                    cview = c_sbs[i][:].rearrange("p (b d) -> p b d", b=nb)
                    nc.vector.tensor_tensor(
                        out=prod[:, poff : poff + nb * KP1 * D],
                        in0=Tembs[i][:],
                        in1=cview.unsqueeze(2).broadcast_to([P, nb, KP1, D]),
                        op=mybir.AluOpType.mult,
                    )
                    poff += nb * KP1 * D
                seg = gnb * KP1
                f64 = wpool.tile([P, seg * 64], mybir.dt.bfloat16, tag="f64")
                pv = prod[:].rearrange("p (s d) -> p s d", d=D)
                nc.vector.tensor_tensor(
                    out=f64[:],
                    in0=pv[:, :, 0:64],
                    in1=pv[:, :, 64:128],
                    op=mybir.AluOpType.add,
                )
                f32 = wpool.tile([P, seg * 32], mybir.dt.bfloat16, tag="f32")
                fv = f64[:].rearrange("p (s d) -> p s d", d=64)
                nc.vector.tensor_tensor(
                    out=f32[:],
                    in0=fv[:, :, 0:32],
                    in1=fv[:, :, 32:64],
                    op=mybir.AluOpType.add,
                )
                f16 = wpool.tile([P, seg * 16], mybir.dt.bfloat16, tag="f16")
                gv = f32[:].rearrange("p (s d) -> p s d", d=32)
                nc.vector.tensor_tensor(
                    out=f16[:],
                    in0=gv[:, :, 0:16],
                    in1=gv[:, :, 16:32],
                    op=mybir.AluOpType.add,
                )
                doff = cbo[g[0]] * KP1
                nc.vector.tensor_reduce(
                    out=all_dots[:, doff : doff + seg],
                    in_=f16[:].rearrange("p (s d) -> p s d", d=16),
                    axis=mybir.AxisListType.X,
                    op=mybir.AluOpType.add,
                )

            # One tail pass: log-sigmoid of all dots (0.1 rescales the ctx
            # sum to a mean); Ln's accum_out emits per-partition sums.
            sig = cpool.tile([P, NBLK * KP1], mybir.dt.float32)
            nc.scalar.activation(
                out=sig[:],
                in_=all_dots[:],
                func=mybir.ActivationFunctionType.Sigmoid,
                scale=1.0 / W,
            )
            ls = cpool.tile([P, NBLK * KP1], mybir.dt.float32)
            nc.scalar.activation(
                out=ls[:],
                in_=sig[:],
                func=mybir.ActivationFunctionType.Ln,
                accum_out=acc[:, 0:1],
            )

            nc.sync.dma_start(out=out_d[:], in_=acc[:])
            if dots_d is not None:
                nc.sync.dma_start(out=dots_d[:], in_=all_dots[:])
    nc.compile()
    return nc


def _pack_indices(context, target, noise):
    """Per-core [P, 336] int32 index matrices in gather layout."""
    ctx_r = np.ascontiguousarray(context, dtype=np.int32).reshape(NCORES, NBLK, P, W)
    tgt_r = np.ascontiguousarray(target, dtype=np.int32).reshape(NCORES, NBLK, P)
    noi_r = np.ascontiguousarray(noise, dtype=np.int32).reshape(NCORES, NBLK, P, K)
    cbo = np.cumsum([0] + CHUNKS).tolist()
    idxs = []
    for n in range(NCORES):
        cols = []
        for i, nb in enumerate(CHUNKS):
            b0 = cbo[i]
            # ctx cols u-major: col (u, b) partition p = context[block b0+b,
            # sample p, word u]
            csg = ctx_r[n, b0 : b0 + nb]  # [nb, P, W]
            cols.append(csg.transpose(2, 0, 1).reshape(W * nb, P).T)
            # emb cols block-major: col (b, j) = [tgt, noise] for sample p
            esg = np.concatenate(
                [tgt_r[n, b0 : b0 + nb, :, None], noi_r[n, b0 : b0 + nb]], axis=2
            )  # [nb, P, 11]
            cols.append(esg.transpose(0, 2, 1).reshape(nb * KP1, P).T)
        idxs.append(np.ascontiguousarray(np.concatenate(cols, axis=1)))
    return idxs


def kernel(context, target, noise, emb_w, ctx_w):
    global _LAST_RESULTS
    import os
    import sys

    for p in ("/root/.axon_site/_ro/trn_rl_repo", "/opt/trn_rl_repo"):
        if p not in sys.path:
            sys.path.insert(0, p)
    import ml_dtypes

    from concourse.bass_utils import run_bass_kernel_spmd

    context = np.asarray(context)
    target = np.asarray(target)
    noise = np.asarray(noise)
    bf16 = ml_dtypes.bfloat16
    emb_w = np.ascontiguousarray(np.asarray(emb_w, dtype=np.float32).astype(bf16))
    ctx_w = np.ascontiguousarray(np.asarray(ctx_w, dtype=np.float32).astype(bf16))

    debug_dots = bool(os.environ.get("KERNEL_DEBUG_DOTS"))
    nc = _build_bass(V, debug_dots=debug_dots)
    idxs = _pack_indices(context, target, noise)
    ident = np.eye(P, dtype=np.float32).astype(bf16)
    in_maps = [
        {"idx": idxs[n], "ident": ident, "ctx_w": ctx_w, "emb_w": emb_w}
        for n in range(NCORES)
    ]
    tmpdir = os.environ.get("KERNEL_TMPDIR") or None
    res = run_bass_kernel_spmd(nc, in_maps, list(range(NCORES)), tmpdir=tmpdir)
    _LAST_RESULTS = res
    total = sum(
        float(np.sum(np.asarray(r["out"], dtype=np.float64))) for r in res.results
    )
    return np.float32(-total / B)


# revision 30
# speedup vs baseline: 1.0595x; 1.0595x over previous
"""CBOW negative-sampling loss kernel for 8 Trainium2 NeuronCores.

Math (faithful to the reference, including its [B]+[B,1] broadcast bug):
    c_b   = mean_w ctx_w[context[b, w]]               # [D]
    pos_b = log_sigmoid(emb_w[target[b]] . c_b)
    neg_b = sum_k log_sigmoid(emb_w[noise[b, k]] . c_b)
    out   = -(mean_b pos_b + mean_b neg_b) = -(sum_b (pos_b + neg_b)) / B

Strategy: shard B across the 8 cores (2048 samples each); tables cast to bf16
on the host (halves the random-gather HBM traffic; the dots are ~1e-4 so bf16
is far inside the fp32 reference envelope). Per core, 16 blocks of 128
samples (partition = sample-in-block), processed as 9 chunks of 1-2 blocks.
Per chunk two indirect gathers run on the Pool SWDGE queue (small enough that
the SWDGE descriptor ring never stalls the Pool engine; tiny first chunks
prime the compute pipeline):
  - ctx rows land [word u][block b][D]; context pooling is 10
    PSUM-accumulating identity matmuls per chunk (constant lhsT, each c_ps
    tile owns a full 2KB PSUM bank); the scalar engine downcasts c to bf16.
  - target+noise rows land [block][tgt, noise*10][D].
Dots run on DVE: one bf16 multiply per chunk against broadcast c (2x mode),
then per chunk-group a chain of 2x tensor_tensor folds (128->64->32->16) and
a short 1x tensor_reduce to fp32 (the fold chain replaces most of the
1x-only tensor_reduce). Sigmoid/Ln activation tables are preloaded at t=0 by
dummy activations so the tail pass (Sigmoid(x/10) + Ln with accum_out giving
per-partition sums) costs ~1us. The host sums the per-core partials and
scales by -1/B.
"""

import numpy as np

V, D = 100000, 128
B, W, K = 16384, 10, 10
NCORES = 8
P = 128
B_LOCAL = B // NCORES  # 2048
NBLK = B_LOCAL // P  # 16 blocks of 128 samples
KP1 = K + 1  # 11 emb rows per sample

# Chunks = gather granularity (blocks per indirect-DMA pair).
CHUNKS = [1, 1, 2, 2, 2, 2, 2, 2, 2]
# Fold groups (indices into CHUNKS): folds/reduce batch several chunks to
# amortize DVE instruction overhead.
GROUPS = [(0, 1, 2), (3, 4), (5, 6), (7, 8)]
assert sum(CHUNKS) == NBLK

_LAST_RESULTS = None  # test harness introspection (exec_time_ns etc.)


def _build_bass(vocab, debug_dots=False):
    import concourse.bass as bass
    import concourse.tile as tile
    from concourse import bacc, mybir

    total_cols = NBLK * W + NBLK * KP1  # 336
    nc = bacc.Bacc(None, target_bir_lowering=False)
    dots_d = (
        nc.declare_dram_parameter(
            "dots", [P, NBLK * KP1], mybir.dt.float32, isOutput=True
        )
        if debug_dots
        else None
    )
    idx_d = nc.declare_dram_parameter(
        "idx", [P, total_cols], mybir.dt.int32, isOutput=False
    )
    ident_d = nc.declare_dram_parameter(
        "ident", [P, P], mybir.dt.bfloat16, isOutput=False
    )
    ctx_w_d = nc.declare_dram_parameter(
        "ctx_w", [vocab, D], mybir.dt.bfloat16, isOutput=False
    )
    emb_w_d = nc.declare_dram_parameter(
        "emb_w", [vocab, D], mybir.dt.bfloat16, isOutput=False
    )
    out_d = nc.declare_dram_parameter("out", [P, 1], mybir.dt.float32, isOutput=True)

    cbo = np.cumsum([0] + CHUNKS).tolist()  # chunk block offsets

    with tile.TileContext(nc) as tc:
        with (
            tc.tile_pool(name="const", bufs=1) as cpool,
            tc.tile_pool(name="work", bufs=2) as wpool,
            tc.tile_pool(name="psum", bufs=3, space="PSUM") as ppool,
        ):
            idx_sb = cpool.tile([P, total_cols], mybir.dt.int32)
            nc.sync.dma_start(out=idx_sb[:], in_=idx_d[:])
            ident_sb = cpool.tile([P, P], mybir.dt.bfloat16)
            nc.sync.dma_start(out=ident_sb[:], in_=ident_d[:])
            all_dots = cpool.tile([P, NBLK * KP1], mybir.dt.float32)
            acc = cpool.tile([P, 1], mybir.dt.float32)

            # Preload Sigmoid/Ln activation tables at t=0 (no deps) so the
            # tail pass doesn't pay two serial ~1.3us table loads.
            dummy = cpool.tile([P, 8], mybir.dt.float32)
            nc.vector.memset(dummy[:], 1.0)
            nc.scalar.activation(
                out=dummy[:], in_=dummy[:], func=mybir.ActivationFunctionType.Sigmoid
            )
            nc.scalar.activation(
                out=dummy[:], in_=dummy[:], func=mybir.ActivationFunctionType.Ln
            )

            # Per-chunk persistent tiles (no recycling -> no false deps).
            c_sbs, Tctxs, Tembs = [], [], []
            for i, nb in enumerate(CHUNKS):
                c_sb = cpool.tile([P, nb * D], mybir.dt.bfloat16, tag=f"c{i}")
                c_sbs.append(c_sb)
                Tctx = cpool.tile([P, W * nb * D], mybir.dt.bfloat16, tag=f"C{i}")
                Tctxs.append(Tctx)
                Temb = cpool.tile([P, nb * KP1 * D], mybir.dt.bfloat16, tag=f"T{i}")
                Tembs.append(Temb)
            prods = []
            for gi, g in enumerate(GROUPS):
                gnb = sum(CHUNKS[i] for i in g)
                prod = wpool.tile(
                    [P, gnb * KP1 * D], mybir.dt.bfloat16, tag=f"prod{gi}"
                )
                prods.append(prod)

            # All gathers up front; the SWDGE queue drains them back to back.
            col = 0
            for i, nb in enumerate(CHUNKS):
                ctx_cols = W * nb
                emb_cols = KP1 * nb
                nc.gpsimd.indirect_dma_start(
                    out=Tctxs[i][:],
                    out_offset=None,
                    in_=ctx_w_d[:],
                    in_offset=bass.IndirectOffsetOnAxis(
                        ap=idx_sb[:, col : col + ctx_cols], axis=0
                    ),
                )
                col += ctx_cols
                nc.gpsimd.indirect_dma_start(
                    out=Tembs[i][:],
                    out_offset=None,
                    in_=emb_w_d[:],
                    in_offset=bass.IndirectOffsetOnAxis(
                        ap=idx_sb[:, col : col + emb_cols], axis=0
                    ),
                )
                col += emb_cols

            # Context pooling: 10 PSUM-accumulating identity matmuls per
            # chunk (c[s, :] = sum_u Tctx[s, slot u]); constant lhsT.
            for i, nb in enumerate(CHUNKS):
                # Full 2KB PSUM bank per tile: matmul start=True resets at
                # bank granularity, so tiles must never share a bank.
                c_ps = ppool.tile([P, 4 * D], mybir.dt.float32, tag="cps")
                for u in range(W):
                    nc.tensor.matmul(
                        c_ps[:, : nb * D],
                        lhsT=ident_sb[:],
                        rhs=Tctxs[i][:, u * nb * D : (u + 1) * nb * D],
                        start=(u == 0),
                        stop=(u == W - 1),
                    )
                nc.scalar.activation(
                    out=c_sbs[i][:],
                    in_=c_ps[:, : nb * D],
                    func=mybir.ActivationFunctionType.Copy,
                )

            # Dots: multiply per chunk (2x), fold chain + reduce per group.
            for gi, g in enumerate(GROUPS):
                prod = prods[gi]
                gnb = sum(CHUNKS[i] for i in g)
                poff = 0
                for i in g:
                    nb = CHUNKS[i]
                    cview = c_sbs[i][:].rearrange("p (b d) -> p b d", b=nb)
                    nc.vector.tensor_tensor(
                        out=prod[:, poff : poff + nb * KP1 * D],
                        in0=Tembs[i][:],
                        in1=cview.unsqueeze(2).broadcast_to([P, nb, KP1, D]),
                        op=mybir.AluOpType.mult,
                    )
                    poff += nb * KP1 * D
                seg = gnb * KP1
                f64 = wpool.tile([P, seg * 64], mybir.dt.bfloat16, tag="f64")
                pv = prod[:].rearrange("p (s d) -> p s d", d=D)
                nc.vector.tensor_tensor(
                    out=f64[:],
                    in0=pv[:, :, 0:64],
                    in1=pv[:, :, 64:128],
                    op=mybir.AluOpType.add,
                )
                f32 = wpool.tile([P, seg * 32], mybir.dt.bfloat16, tag="f32")
                fv = f64[:].rearrange("p (s d) -> p s d", d=64)
                nc.vector.tensor_tensor(
                    out=f32[:],
                    in0=fv[:, :, 0:32],
                    in1=fv[:, :, 32:64],
                    op=mybir.AluOpType.add,
                )
                f16 = wpool.tile([P, seg * 16], mybir.dt.bfloat16, tag="f16")
                gv = f32[:].rearrange("p (s d) -> p s d", d=32)
                nc.vector.tensor_tensor(
                    out=f16[:],
                    in0=gv[:, :, 0:16],
                    in1=gv[:, :, 16:32],
                    op=mybir.AluOpType.add,
                )
                doff = cbo[g[0]] * KP1
                nc.vector.tensor_reduce(
                    out=all_dots[:, doff : doff + seg],
                    in_=f16[:].rearrange("p (s d) -> p s d", d=16),
                    axis=mybir.AxisListType.X,
                    op=mybir.AluOpType.add,
                )

            # One tail pass: log-sigmoid of all dots (0.1 rescales the ctx
            # sum to a mean); Ln's accum_out emits per-partition sums.
            sig = cpool.tile([P, NBLK * KP1], mybir.dt.float32)
            nc.scalar.activation(
                out=sig[:],
                in_=all_dots[:],
                func=mybir.ActivationFunctionType.Sigmoid,
                scale=1.0 / W,
            )
            ls = cpool.tile([P, NBLK * KP1], mybir.dt.float32)
            nc.scalar.activation(
                out=ls[:],
                in_=sig[:],
                func=mybir.ActivationFunctionType.Ln,
                accum_out=acc[:, 0:1],
            )

            nc.sync.dma_start(out=out_d[:], in_=acc[:])
            if dots_d is not None:
                nc.sync.dma_start(out=dots_d[:], in_=all_dots[:])
    nc.compile()
    return nc


def _pack_indices(context, target, noise):
    """Per-core [P, 336] int32 index matrices in gather layout."""
    ctx_r = np.ascontiguousarray(context, dtype=np.int32).reshape(NCORES, NBLK, P, W)
    tgt_r = np.ascontiguousarray(target, dtype=np.int32).reshape(NCORES, NBLK, P)
    noi_r = np.ascontiguousarray(noise, dtype=np.int32).reshape(NCORES, NBLK, P, K)
    cbo = np.cumsum([0] + CHUNKS).tolist()
    idxs = []
    for n in range(NCORES):
        cols = []
        for i, nb in enumerate(CHUNKS):
            b0 = cbo[i]
            # ctx cols u-major: col (u, b) partition p = context[block b0+b,
            # sample p, word u]
            csg = ctx_r[n, b0 : b0 + nb]  # [nb, P, W]
            cols.append(csg.transpose(2, 0, 1).reshape(W * nb, P).T)
            # emb cols block-major: col (b, j) = [tgt, noise] for sample p
            esg = np.concatenate(
                [tgt_r[n, b0 : b0 + nb, :, None], noi_r[n, b0 : b0 + nb]], axis=2
            )  # [nb, P, 11]
            cols.append(esg.transpose(0, 2, 1).reshape(nb * KP1, P).T)
        idxs.append(np.ascontiguousarray(np.concatenate(cols, axis=1)))
    return idxs


def kernel(context, target, noise, emb_w, ctx_w):
    global _LAST_RESULTS
    import os
    import sys

    for p in ("/root/.axon_site/_ro/trn_rl_repo", "/opt/trn_rl_repo"):
        if p not in sys.path:
            sys.path.insert(0, p)
    import ml_dtypes

    from concourse.bass_utils import run_bass_kernel_spmd

    context = np.asarray(context)
    target = np.asarray(target)
    noise = np.asarray(noise)
    bf16 = ml_dtypes.bfloat16
    emb_w = np.ascontiguousarray(np.asarray(emb_w, dtype=np.float32).astype(bf16))
    ctx_w = np.ascontiguousarray(np.asarray(ctx_w, dtype=np.float32).astype(bf16))

    debug_dots = bool(os.environ.get("KERNEL_DEBUG_DOTS"))
    nc = _build_bass(V, debug_dots=debug_dots)
    idxs = _pack_indices(context, target, noise)
    ident = np.eye(P, dtype=np.float32).astype(bf16)
    in_maps = [
        {"idx": idxs[n], "ident": ident, "ctx_w": ctx_w, "emb_w": emb_w}
        for n in range(NCORES)
    ]
    tmpdir = os.environ.get("KERNEL_TMPDIR") or None
    res = run_bass_kernel_spmd(nc, in_maps, list(range(NCORES)), tmpdir=tmpdir)
    _LAST_RESULTS = res
    total = sum(
        float(np.sum(np.asarray(r["out"], dtype=np.float64))) for r in res.results
    )
    return np.float32(-total / B)
